# revision 36
# baseline (speedup 1.0000x reference)
"""Trainium2 Bass kernel for CausalWaveletFieldAttention.

Full-input contract: kernel(**inputs) takes the complete (unsharded) numpy
inputs and returns the full [8, 2048, 1024] float32 output.

Sharding: pure data-parallel over batch B=8 -> one batch element per
NeuronCore (8 cores), zero collectives (the head-coupling einsum mixes heads
within a batch element only).

Per-core pipeline (x pre-transposed to feature-major on host, bf16 compute,
fp32 PSUM accumulation, fp8 DoubleRow for the k and gate projections):
  1. k = x8 @ Wk8 (fp8 DoubleRow), k2 = Square(k + bk) (ScalarE), per-head
     sums via a replicating selector matmul -> kmag_rep[128, N] directly.
  2. v = x @ Wv.T with output channels in d-major order (c~ = d*16 + h);
     field = (v + bv) * kmag (fused DVE op), channel-major [c~, n].
  3. causal multi-scale conv split two ways:
       - offsets <= 128 (15 of the 22): dense block-Toeplitz matmuls in
         token-major space. field is PE-transposed to [token, c~] tiles;
         for each 128-token output tile T and m in {0,1}, a per-head
         [128,128] Toeplitz stationary G_m[h] (HOST-built from
         softmax(scale_gain) and the D4 taps) multiplies field_tm[T-m],
         accumulating all 15 offsets in 2 passes/head instead of 15.
       - offsets > 128 (7): per-partition scalar MACs on VectorE
         (free-axis shifts), accumulating into the transposed-back acc.
  4. head coupling: block-diagonal I_8 (x) softmax(C)^T stationary
     (host-built) -> one [128,128] matmul per channel tile.
  5. gate = Sigmoid(x8 @ Wg8 + 2.0) (fp8 DoubleRow, d-major, per-segment).
  6. out = (coupled * gate).T @ Wo.T with gated [c~,n] chunks stationary so
     the output lands token-major (bf16) for the DMA out.
"""

import os
import sys

import numpy as np

# recover wedged NeuronCores from a previously killed process
os.environ.setdefault("NEURON_RT_RESET_CORES", "1")

for _p in ("/opt/trn_rl_repo", "/root/.axon_site/_ro/trn_rl_repo"):
    if _p not in sys.path:
        sys.path.append(_p)

import ml_dtypes  # noqa: E402
import concourse.bass as bass  # noqa: E402
import concourse.tile as tile  # noqa: E402
from concourse import bacc, mybir  # noqa: E402
from concourse import bass_utils  # noqa: E402

BF16 = mybir.dt.bfloat16
F32 = mybir.dt.float32
FP8 = mybir.dt.float8e4
NP_BF16 = ml_dtypes.bfloat16
NP_FP8 = ml_dtypes.float8_e4m3

B, N, D = 8, 2048, 1024
H, HD = 16, 64
S = 11  # scales
NCORES = 8
P = 128  # partitions
CH = D // P  # 8 channel chunks
NT = N // P  # 16 token tiles
NCK = N // 512  # 4 free-dim 512 chunks

D4 = np.array(
    [0.4829629131445341, 0.8365163037378079, 0.2241438680420134, -0.1294095225512604],
    dtype=np.float64,
)

# Distinct causal time offsets (3-t)*2^j < N, and the [n_offsets, S] map s.t.
# w[o, h] = sum_j A_MAP[o, j] * softmax_gains[j, h]
_offs = sorted({(3 - t) * (1 << j) for j in range(S) for t in range(4)} & set(range(N)))
OFFSETS = list(_offs)
NOFF = len(OFFSETS)  # 22
A_MAP = np.zeros((NOFF, S), dtype=np.float64)
for j in range(S):
    for t in range(4):
        o = (3 - t) * (1 << j)
        if o < N:
            A_MAP[OFFSETS.index(o), j] += D4[t]

# offsets <= 128 are covered exactly by Toeplitz blocks m in {0, 1};
# larger offsets run as shifted per-partition MACs on VectorE.
DVE_OFFS = [oi for oi, o in enumerate(OFFSETS) if o > 128]

# d-major channel permutation: c~ -> original feature h*64 + d
PERM = np.array([(c % H) * HD + c // H for c in range(D)], dtype=np.int64)

_CACHE = {}


def _build_program(iters=1, ob_zero=False):
    nc = bacc.Bacc("TRN2", target_bir_lowering=False, debug=False, num_devices=NCORES)

    # ---- I/O ----
    x_cm = nc.dram_tensor("x_cm", [D, N], BF16, kind="ExternalInput")
    # fp8 DoubleRow operands: contraction index c = 256*ic + 2*ki + j
    # laid out as [ki, ic, j, .]
    x8_d = nc.dram_tensor("x8", [P, 4, 2, N], FP8, kind="ExternalInput")
    wk8_d = nc.dram_tensor("wk8", [P, 4, 2, D], FP8, kind="ExternalInput")
    wv_d = nc.dram_tensor("wv", [D, D], BF16, kind="ExternalInput")  # [c_in, c~]
    wg8_d = nc.dram_tensor("wg8", [P, 4, 2, D], FP8, kind="ExternalInput")
    wo_d = nc.dram_tensor("wo", [D, D], BF16, kind="ExternalInput")  # [c~, f]
    bk_d = nc.dram_tensor("bk", [P, CH], F32, kind="ExternalInput")
    bv_d = nc.dram_tensor("bv", [P, CH], F32, kind="ExternalInput")
    bg_d = nc.dram_tensor("bg", [P, CH], F32, kind="ExternalInput")
    ob_d = nc.dram_tensor("ob", [P, D], F32, kind="ExternalInput")  # out_b row-bcast
    # host-built toeplitz stationaries [p, m, h, col] and DVE conv weights
    gt_d = nc.dram_tensor("gt", [P, 2, H, P], BF16, kind="ExternalInput")
    wrep_d = nc.dram_tensor("wrep", [P, NOFF], F32, kind="ExternalInput")
    gcpl_d = nc.dram_tensor("gcpl", [P, P], BF16, kind="ExternalInput")
    y_d = nc.dram_tensor("y", [N, D], BF16, kind="ExternalOutput")

    # ---- constants (embedded in NEFF) ----
    i128_d = nc.inline_tensor(np.eye(P, dtype=NP_BF16), "i128")
    # kmag selector: sums k2 over each head's 64 partitions AND replicates
    # the result to all 128 kmag partitions (head = row % 16, d-major)
    sel2 = np.zeros((P, CH, P), dtype=NP_BF16)
    for kc in range(CH):
        for p in range(P):
            h = 2 * kc + p // HD
            for po in range(h, P, H):
                sel2[p, kc, po] = 1
    sel2_d = nc.inline_tensor(np.ascontiguousarray(sel2), "sel2")

    import contextlib
    with tile.TileContext(nc) as tc, contextlib.ExitStack() as _st:
      for _it in range(iters):
          with contextlib.ExitStack() as _it_st:
              ec = _it_st.enter_context
              cpool = ec(tc.tile_pool(name="consts", bufs=1))
              xpool = ec(tc.tile_pool(name="xpool", bufs=1))
              x8pool = ec(tc.tile_pool(name="x8p", bufs=1))
              wf8pool = ec(tc.tile_pool(name="wf8p", bufs=1))
              wvpool = ec(tc.tile_pool(name="wvp", bufs=1))
              wopool = ec(tc.tile_pool(name="wop", bufs=1))
              fpool = ec(tc.tile_pool(name="field", bufs=1))
              apool = ec(tc.tile_pool(name="accp", bufs=1))
              ftmpool = ec(tc.tile_pool(name="ftm", bufs=3))
              atmpool = ec(tc.tile_pool(name="atm", bufs=4))
              gpool = ec(tc.tile_pool(name="gring", bufs=2))
              k2pool = ec(tc.tile_pool(name="k2p", bufs=1))
              ypool = ec(tc.tile_pool(name="ystg", bufs=1))
              pspool = ec(tc.tile_pool(name="psum", bufs=4, space="PSUM"))
              tppool = ec(tc.tile_pool(name="psum_tp", bufs=2, space="PSUM"))
              # ============ big streaming inputs first (head latency) ======
              x_sb = xpool.tile([P, CH, N], BF16)
              x8_sb = x8pool.tile([P, 4, 2, N], FP8)
              wf8 = wf8pool.tile([P, 4, 2, 2 * D], FP8)
              nc.sync.dma_start(out=wf8[:, :, :, 0:D], in_=wk8_d[:, :, :, :])
              for sq in range(NCK):
                  nsq = 512 * sq
                  nc.sync.dma_start(out=x8_sb[:, :, :, nsq:nsq + 512],
                                    in_=x8_d[:, :, :, nsq:nsq + 512])
                  for ic in range(CH):
                      nc.sync.dma_start(
                          out=x_sb[:, ic, nsq:nsq + 512],
                          in_=x_cm[P * ic:P * (ic + 1), nsq:nsq + 512])

              # ============ small parameter loads ============
              i128_sb = cpool.tile([P, P], BF16)
              nc.gpsimd.dma_start(out=i128_sb[:, :], in_=i128_d[:, :])
              sel2_sb = cpool.tile([P, CH, P], BF16)
              nc.gpsimd.dma_start(out=sel2_sb[:, :, :], in_=sel2_d[:, :, :])
              gt_sb = cpool.tile([P, 2, H, P], BF16)
              nc.gpsimd.dma_start(out=gt_sb[:, :, :, :], in_=gt_d[:, :, :, :])
              wrep_sb = cpool.tile([P, NOFF], F32)
              nc.gpsimd.dma_start(out=wrep_sb[:, :], in_=wrep_d[:, :])
              gcpl_sb = cpool.tile([P, P], BF16)
              nc.gpsimd.dma_start(out=gcpl_sb[:, :], in_=gcpl_d[:, :])
              bk_sb = cpool.tile([P, CH], F32)
              nc.gpsimd.dma_start(out=bk_sb[:, :], in_=bk_d[:, :])
              bv_sb = cpool.tile([P, CH], F32)
              nc.gpsimd.dma_start(out=bv_sb[:, :], in_=bv_d[:, :])
              bg_sb = cpool.tile([P, CH], F32)
              nc.gpsimd.dma_start(out=bg_sb[:, :], in_=bg_d[:, :])
              if not ob_zero:
                  ob_sb = cpool.tile([P, D], F32)
                  nc.gpsimd.dma_start(out=ob_sb[:, :], in_=ob_d[:, :])

              # ============ k phase helper: kmag_rep[128, N] ============
              kmag_rep = cpool.tile([P, N], BF16)

              def kphase(s):
                  ns = 512 * s
                  km_ps = pspool.tile([P, 512], F32, tag="mm")
                  for kc in range(CH):
                      ps = pspool.tile([P, 512], F32, tag="mm")
                      for ic in range(4):
                          nc.tensor.matmul(
                              ps[:, :],
                              lhsT=wf8[:, ic, :, P * kc:P * (kc + 1)],
                              rhs=x8_sb[:, ic, :, ns:ns + 512],
                              perf_mode=mybir.MatmulPerfMode.DoubleRow,
                              start=(ic == 0), stop=(ic == 3),
                          )
                      k2 = k2pool.tile([P, 512], BF16, tag="k2")
                      nc.scalar.activation(
                          out=k2[:, :], in_=ps[:, :],
                          func=mybir.ActivationFunctionType.Square,
                          bias=bk_sb[:, kc:kc + 1], scale=1.0,
                      )
                      nc.tensor.matmul(
                          km_ps[:, :],
                          lhsT=sel2_sb[:, kc, :], rhs=k2[:, :],
                          start=(kc == 0), stop=(kc == CH - 1),
                      )
                  nc.scalar.activation(
                      out=kmag_rep[:, ns:ns + 512], in_=km_ps[:, :],
                      func=mybir.ActivationFunctionType.Sqrt,
                  )

              # ============ weights for v ============
              wv_sb = wvpool.tile([P, CH, D], BF16)
              for ic in range(CH):
                  nc.sync.dma_start(out=wv_sb[:, ic, :], in_=wv_d[P * ic:P * (ic + 1), :])
              nc.sync.dma_start(out=wf8[:, :, :, D:2 * D], in_=wg8_d[:, :, :, :])
              wo_sb = wopool.tile([P, CH, D], BF16)
              for ic in range(CH):
                  nc.sync.dma_start(out=wo_sb[:, ic, :], in_=wo_d[P * ic:P * (ic + 1), :])

              field = fpool.tile([P, CH, N], BF16)
              acc = apool.tile([P, CH, N], BF16)
              ftm = [None] * NT   # token-major field tiles (ring)
              atm = [None] * NT   # token-major conv acc tiles (ring, d-major)
              gate_ring = [None] * NCK

              def tail(s):
                  ns = 512 * s
                  g_sb = gate_ring[s]
                  for vc in range(CH):
                      ps = pspool.tile([P, 512], F32, tag="mm")
                      nc.tensor.matmul(
                          ps[:, :], lhsT=gcpl_sb[:, :], rhs=acc[:, vc, ns:ns + 512],
                          start=True, stop=True,
                      )
                      # gated = coupled * gate, in place into the gate ring
                      nc.vector.tensor_mul(
                          g_sb[:, vc, :], ps[:, :], g_sb[:, vc, :],
                      )
                  for nt in range(4 * s, 4 * s + 4):
                      nl = P * (nt - 4 * s)
                      ystg = ypool.tile([P, D], BF16, tag="y")
                      for fch in range(2):
                          fs = 512 * fch
                          ps = pspool.tile([P, 512], F32, tag="mm")
                          for vc in range(CH):
                              nc.tensor.matmul(
                                  ps[:, :],
                                  lhsT=g_sb[:, vc, nl:nl + P],
                                  rhs=wo_sb[:, vc, fs:fs + 512],
                                  start=(vc == 0), stop=(vc == CH - 1),
                              )
                          if ob_zero:
                              nc.scalar.activation(
                                  out=ystg[:, fs:fs + 512], in_=ps[:, :],
                                  func=mybir.ActivationFunctionType.Copy,
                              )
                          else:
                              nc.vector.tensor_add(
                                  ystg[:, fs:fs + 512], ps[:, :],
                                  ob_sb[:, fs:fs + 512],
                              )
                      nc.sync.dma_start(out=y_d[P * nt:P * (nt + 1), :], in_=ystg[:, :])

              for s in range(NCK):
                  ns = 512 * s
                  kphase(s)
                  # ---- v matmuls + field (channel-major) for this segment --
                  for vc in range(CH):
                      ps = pspool.tile([P, 512], F32, tag="mm")
                      for ic in range(CH):
                          nc.tensor.matmul(
                              ps[:, :],
                              lhsT=wv_sb[:, ic, P * vc:P * (vc + 1)],
                              rhs=x_sb[:, ic, ns:ns + 512],
                              start=(ic == 0), stop=(ic == CH - 1),
                          )
                      nc.vector.scalar_tensor_tensor(
                          out=field[:, vc, ns:ns + 512],
                          in0=ps[:, :], scalar=bv_sb[:, vc:vc + 1],
                          in1=kmag_rep[:, ns:ns + 512],
                          op0=mybir.AluOpType.add, op1=mybir.AluOpType.mult,
                      )
                  # ---- per 128-token tile: transpose + toeplitz conv ----
                  # software-pipelined: ttm(T+1) and toep(T) interleave so
                  # the PSUM->SBUF drains overlap PE work
                  def ttm(T):
                      ts0 = P * T
                      pst = pspool.tile([P, HD, H], BF16, tag="mm")
                      for vc in range(CH):
                          nc.tensor.transpose(
                              out=pst[:, CH * vc:CH * (vc + 1), :],
                              in_=field[:, vc, ts0:ts0 + P],
                              identity=i128_sb[:, :],
                          )
                      f_tm = ftmpool.tile([P, HD, H], BF16, tag="ftm")
                      ftm[T] = f_tm
                      nc.scalar.activation(
                          out=f_tm[:, :, :], in_=pst[:, :, :],
                          func=mybir.ActivationFunctionType.Copy,
                      )

                  def toep(T):
                      # toeplitz: psum cols head-major [h, d]
                      ps2 = tppool.tile([P, H, HD], F32, tag="tp")
                      for h in range(H):
                          ms = [0] if T == 0 else [0, 1]
                          for m in ms:
                              nc.tensor.matmul(
                                  ps2[:, h, :],
                                  lhsT=gt_sb[:, m, h, :],
                                  rhs=ftm[T - m][:, :, h],
                                  start=(m == ms[0]), stop=(m == ms[-1]),
                              )
                      a_tm = atmpool.tile([P, HD, H], BF16, tag="atm")
                      atm[T] = a_tm
                      # reorder (h, d) -> (d, h) during the PSUM drain
                      nc.scalar.activation(
                          out=a_tm[:, :, :],
                          in_=ps2[:, :, :].transpose([0, 2, 1]),
                          func=mybir.ActivationFunctionType.Copy,
                      )

                  ttm(4 * s)
                  for T in range(4 * s, 4 * s + 4):
                      if T + 1 < 4 * s + 4:
                          ttm(T + 1)
                      toep(T)
                  # ---- transpose back to channel-major acc ----
                  for vc in range(CH):
                      pstb = pspool.tile([P, 512], BF16, tag="mm")
                      for ti in range(4):
                          T = 4 * s + ti
                          nc.tensor.transpose(
                              out=pstb[:, P * ti:P * (ti + 1)],
                              in_=atm[T][:, CH * vc:CH * (vc + 1), :],
                              identity=i128_sb[:, :],
                          )
                      nc.scalar.activation(
                          out=acc[:, vc, ns:ns + 512], in_=pstb[:, :],
                          func=mybir.ActivationFunctionType.Copy,
                      )
                  # ---- large offsets: shifted MACs on VectorE (the Pool
                  # engine rejects TensorScalarPtr on real TRN2) ----
                  def macs(s2, vcs):
                      ns2 = 512 * s2
                      for vc in vcs:
                          for oi in DVE_OFFS:
                              o = OFFSETS[oi]
                              lo = max(ns2, o)
                              if lo < ns2 + 512:
                                  nc.vector.scalar_tensor_tensor(
                                      out=acc[:, vc, lo:ns2 + 512],
                                      in0=field[:, vc, lo - o:ns2 + 512 - o],
                                      scalar=wrep_sb[:, oi:oi + 1],
                                      in1=acc[:, vc, lo:ns2 + 512],
                                      op0=mybir.AluOpType.mult,
                                      op1=mybir.AluOpType.add,
                                  )
                  macs(s, range(0, 4))
                  # ---- gate for this segment (fp8 DoubleRow) ----
                  g_sb = gpool.tile([P, CH, 512], BF16, tag="g")
                  gate_ring[s] = g_sb
                  for gc in range(CH):
                      ps = pspool.tile([P, 512], F32, tag="mm")
                      for ic in range(4):
                          nc.tensor.matmul(
                              ps[:, :],
                              lhsT=wf8[:, ic, :, D + P * gc:D + P * (gc + 1)],
                              rhs=x8_sb[:, ic, :, ns:ns + 512],
                              perf_mode=mybir.MatmulPerfMode.DoubleRow,
                              start=(ic == 0), stop=(ic == 3),
                          )
                      nc.scalar.activation(
                          out=g_sb[:, gc, :], in_=ps[:, :],
                          func=mybir.ActivationFunctionType.Sigmoid,
                          bias=bg_sb[:, gc:gc + 1], scale=1.0,
                      )
                  macs(s, range(4, CH))
                  if s >= 1:
                      tail(s - 1)
              tail(NCK - 1)

    nc.compile()
    return nc


def _prep_shared(qkv_w, qkv_b, out_w, out_b, gate_w, gate_b, scale_gain, field_coupling):
    perm = PERM
    wk8 = np.ascontiguousarray(
        qkv_w[D:2 * D, :].T.reshape(4, P, 2, D).transpose(1, 0, 2, 3)
        .astype(NP_FP8))
    wv = np.ascontiguousarray(qkv_w[2 * D:3 * D, :][perm, :].T.astype(NP_BF16))
    wg8 = np.ascontiguousarray(
        gate_w[perm, :].T.reshape(4, P, 2, D).transpose(1, 0, 2, 3)
        .astype(NP_FP8))
    wo = np.ascontiguousarray(out_w[:, perm].T.astype(NP_BF16))
    bk = np.ascontiguousarray(qkv_b[D:2 * D].reshape(CH, P).T.astype(np.float32))
    bv = np.ascontiguousarray(qkv_b[2 * D:3 * D][perm].reshape(CH, P).T.astype(np.float32))
    bg = np.ascontiguousarray(gate_b[perm].reshape(CH, P).T.astype(np.float32))
    ob = np.ascontiguousarray(np.broadcast_to(out_b.astype(np.float32), (P, D)))

    # conv weights: w[h, o] = sum_j A_MAP[o, j] * softmax(scale_gain)[j, h]
    sg = scale_gain.astype(np.float64)
    e = np.exp(sg - sg.max(axis=0, keepdims=True))
    gains = e / e.sum(axis=0, keepdims=True)             # [S, H]
    w_all = (A_MAP @ gains).T                            # [H, NOFF]
    wband = np.zeros((H, 130), dtype=np.float64)
    for oi, o in enumerate(OFFSETS):
        if o <= P:
            wband[:, o] = w_all[:, oi]
    pp, cc = np.meshgrid(np.arange(P), np.arange(P), indexing="ij")
    d0 = cc - pp                              # toeplitz block m=0: o = c - p
    g0 = wband[:, np.clip(d0, 0, 129)] * (d0 >= 0)[None]
    d1 = d0 + P                               # block m=1: o = c - p + 128
    g1 = wband[:, np.clip(d1, 0, 129)] * ((d1 >= 1) & (d1 <= P))[None]
    gt = np.stack([g0, g1], axis=0).transpose(2, 0, 1, 3).astype(NP_BF16)
    wrep = np.ascontiguousarray(
        w_all[np.arange(P) % H, :].astype(np.float32))   # [P, NOFF]

    fc = field_coupling.astype(np.float64)
    ec = np.exp(fc - fc.max(axis=-1, keepdims=True))
    csm = ec / ec.sum(axis=-1, keepdims=True)            # [H, H] softmax rows
    gcpl = np.zeros((P, P), dtype=NP_BF16)
    for r in range(CH):
        gcpl[H * r:H * (r + 1), H * r:H * (r + 1)] = csm.T.astype(NP_BF16)

    return {"wk8": wk8, "wv": wv, "wg8": wg8, "wo": wo, "bk": bk,
            "bv": bv, "bg": bg, "ob": ob, "gt": np.ascontiguousarray(gt),
            "wrep": wrep, "gcpl": np.ascontiguousarray(gcpl)}


def _make_in_maps(x, shared):
    in_maps = []
    for b in range(B):
        m = dict(shared)
        xt = x[b].T
        m["x_cm"] = np.ascontiguousarray(xt.astype(NP_BF16))
        m["x8"] = np.ascontiguousarray(
            xt.reshape(4, P, 2, N).transpose(1, 0, 2, 3).astype(NP_FP8))
        in_maps.append(m)
    return in_maps


def kernel(x, qkv_w, qkv_b, out_w, out_b, gate_w, gate_b, scale_gain,
           field_coupling):
    x = np.asarray(x, dtype=np.float32)
    qkv_w = np.asarray(qkv_w, dtype=np.float32)
    qkv_b = np.asarray(qkv_b, dtype=np.float32)
    out_w = np.asarray(out_w, dtype=np.float32)
    out_b = np.asarray(out_b, dtype=np.float32)
    gate_w = np.asarray(gate_w, dtype=np.float32)
    gate_b = np.asarray(gate_b, dtype=np.float32)
    scale_gain = np.asarray(scale_gain, dtype=np.float32)
    field_coupling = np.asarray(field_coupling, dtype=np.float32)

    ob_zero = not np.any(out_b)
    key = ("nc", ob_zero)
    if key not in _CACHE:
        _CACHE[key] = _build_program(ob_zero=ob_zero)
    nc = _CACHE[key]

    shared = _prep_shared(qkv_w, qkv_b, out_w, out_b, gate_w, gate_b,
                          scale_gain, field_coupling)
    in_maps = _make_in_maps(x, shared)

    res = bass_utils.run_bass_kernel_spmd(nc, in_maps, list(range(NCORES)))
    out = np.stack([np.asarray(res.results[b]["y"]).astype(np.float32)
                    for b in range(B)], axis=0)
    return out


# revision 41
# speedup vs baseline: 1.4566x; 1.4566x over previous
"""Trainium2 Bass kernel for CausalWaveletFieldAttention.

Full-input contract: kernel(**inputs) takes the complete (unsharded) numpy
inputs and returns the full [8, 2048, 1024] float32 output.

Sharding: pure data-parallel over batch B=8 -> one batch element per
NeuronCore (8 cores), zero collectives (the head-coupling einsum mixes heads
within a batch element only).

Per-core pipeline (x pre-transposed to feature-major on host, bf16 compute,
fp32 PSUM accumulation, fp8 DoubleRow for the k and gate projections):
  1. k = x8 @ Wk8 (fp8 DoubleRow), k2 = Square(k + bk) (ScalarE), per-head
     sums via a replicating selector matmul -> kmag_rep[128, N] directly.
  2. v = x @ Wv.T with output channels in d-major order (c~ = d*16 + h);
     field = (v + bv) * kmag (fused DVE op), channel-major [c~, n].
  3. causal multi-scale conv split two ways:
       - offsets <= 128 (15 of the 22): dense block-Toeplitz matmuls in
         token-major space. field is PE-transposed to [token, c~] tiles;
         for each 128-token output tile T and m in {0,1}, a per-head
         [128,128] Toeplitz stationary G_m[h] (HOST-built from
         softmax(scale_gain) and the D4 taps) multiplies field_tm[T-m],
         accumulating all 15 offsets in 2 passes/head instead of 15.
       - offsets > 128 (7): per-partition scalar MACs on VectorE
         (free-axis shifts), accumulating into the transposed-back acc.
  4. head coupling: block-diagonal I_8 (x) softmax(C)^T stationary
     (host-built) -> one [128,128] matmul per channel tile.
  5. gate = Sigmoid(x8 @ Wg8 + 2.0) (fp8 DoubleRow, d-major, per-segment).
  6. out = (coupled * gate).T @ Wo.T with gated [c~,n] chunks stationary so
     the output lands token-major (bf16) for the DMA out.
"""

import os
import sys

import numpy as np

# recover wedged NeuronCores from a previously killed process
os.environ.setdefault("NEURON_RT_RESET_CORES", "1")

for _p in ("/opt/trn_rl_repo", "/root/.axon_site/_ro/trn_rl_repo"):
    if _p not in sys.path:
        sys.path.append(_p)

import ml_dtypes  # noqa: E402
import concourse.bass as bass  # noqa: E402
import concourse.tile as tile  # noqa: E402
from concourse import bacc, mybir  # noqa: E402
from concourse import bass_utils  # noqa: E402

BF16 = mybir.dt.bfloat16
F32 = mybir.dt.float32
FP8 = mybir.dt.float8e4
NP_BF16 = ml_dtypes.bfloat16
NP_FP8 = ml_dtypes.float8_e4m3

B, N, D = 8, 2048, 1024
H, HD = 16, 64
S = 11  # scales
NCORES = 8
P = 128  # partitions
CH = D // P  # 8 channel chunks
NT = N // P  # 16 token tiles
NCK = N // 512  # 4 free-dim 512 chunks

D4 = np.array(
    [0.4829629131445341, 0.8365163037378079, 0.2241438680420134, -0.1294095225512604],
    dtype=np.float64,
)

# Distinct causal time offsets (3-t)*2^j < N, and the [n_offsets, S] map s.t.
# w[o, h] = sum_j A_MAP[o, j] * softmax_gains[j, h]
_offs = sorted({(3 - t) * (1 << j) for j in range(S) for t in range(4)} & set(range(N)))
OFFSETS = list(_offs)
NOFF = len(OFFSETS)  # 22
A_MAP = np.zeros((NOFF, S), dtype=np.float64)
for j in range(S):
    for t in range(4):
        o = (3 - t) * (1 << j)
        if o < N:
            A_MAP[OFFSETS.index(o), j] += D4[t]

# offsets <= 384 are covered exactly by Toeplitz blocks m in {0,1,2,3};
# larger offsets run as shifted per-partition MACs on VectorE.
TOEP_M = 4
DVE_OFFS = [oi for oi, o in enumerate(OFFSETS) if o > 384]

# d-major channel permutation: c~ -> original feature h*64 + d
PERM = np.array([(c % H) * HD + c // H for c in range(D)], dtype=np.int64)

_CACHE = {}


def _build_program(iters=1, ob_zero=False):
    nc = bacc.Bacc("TRN2", target_bir_lowering=False, debug=False, num_devices=NCORES)

    # ---- I/O ----
    # fp8 DoubleRow operands: contraction index c = 256*ic + 2*ki + j
    # laid out as [ki, ic, j, .]; x8l/wv8l are the fp8 residuals for the
    # hi/lo split v matmul (x ~ x8 + x8l to ~0.03%)
    x8_d = nc.dram_tensor("x8", [P, 4, 2, N], FP8, kind="ExternalInput")
    x8l_d = nc.dram_tensor("x8l", [P, 4, 2, N], FP8, kind="ExternalInput")
    wk8_d = nc.dram_tensor("wk8", [P, 4, 2, D], FP8, kind="ExternalInput")
    wv8_d = nc.dram_tensor("wv8", [P, 4, 2, D], FP8, kind="ExternalInput")
    wv8l_d = nc.dram_tensor("wv8l", [P, 4, 2, D], FP8, kind="ExternalInput")
    wg8_d = nc.dram_tensor("wg8", [P, 4, 2, D], FP8, kind="ExternalInput")
    wo_d = nc.dram_tensor("wo", [D, D], BF16, kind="ExternalInput")  # [c~, f]
    bk_d = nc.dram_tensor("bk", [P, CH], F32, kind="ExternalInput")
    bv_d = nc.dram_tensor("bv", [P, CH], F32, kind="ExternalInput")
    bg_d = nc.dram_tensor("bg", [P, CH], F32, kind="ExternalInput")
    ob_d = nc.dram_tensor("ob", [P, D], F32, kind="ExternalInput")  # out_b row-bcast
    # host-built toeplitz stationaries [p, m, h, col] and DVE conv weights
    gt_d = nc.dram_tensor("gt", [P, TOEP_M, H, P], BF16, kind="ExternalInput")
    wrep_d = nc.dram_tensor("wrep", [P, NOFF], F32, kind="ExternalInput")
    gcpl_d = nc.dram_tensor("gcpl", [P, P], BF16, kind="ExternalInput")
    y_d = nc.dram_tensor("y", [N, D], BF16, kind="ExternalOutput")

    # ---- constants (embedded in NEFF) ----
    i128_d = nc.inline_tensor(np.eye(P, dtype=NP_BF16), "i128")
    # kmag selector: sums k2 over each head's 64 partitions AND replicates
    # the result to all 128 kmag partitions (head = row % 16, d-major)
    sel2 = np.zeros((P, CH, P), dtype=NP_BF16)
    for kc in range(CH):
        for p in range(P):
            h = 2 * kc + p // HD
            for po in range(h, P, H):
                sel2[p, kc, po] = 1
    sel2_d = nc.inline_tensor(np.ascontiguousarray(sel2), "sel2")

    import contextlib
    with tile.TileContext(nc) as tc, contextlib.ExitStack() as _st:
      for _it in range(iters):
          with contextlib.ExitStack() as _it_st:
              ec = _it_st.enter_context
              cpool = ec(tc.tile_pool(name="consts", bufs=1))
              xpool = ec(tc.tile_pool(name="xpool", bufs=1))
              x8pool = ec(tc.tile_pool(name="x8p", bufs=1))
              wf8pool = ec(tc.tile_pool(name="wf8p", bufs=1))
              wopool = ec(tc.tile_pool(name="wop", bufs=1))
              fpool = ec(tc.tile_pool(name="field", bufs=1))
              apool = ec(tc.tile_pool(name="accp", bufs=1))
              ftmpool = ec(tc.tile_pool(name="ftm", bufs=5))
              atmpool = ec(tc.tile_pool(name="atm", bufs=4))
              gpool = ec(tc.tile_pool(name="gring", bufs=2))
              k2pool = ec(tc.tile_pool(name="k2p", bufs=1))
              ypool = ec(tc.tile_pool(name="ystg", bufs=1))
              pspool = ec(tc.tile_pool(name="psum", bufs=4, space="PSUM"))
              tppool = ec(tc.tile_pool(name="psum_tp", bufs=2, space="PSUM"))
              # ============ big streaming inputs first (head latency) ======
              x8_sb = x8pool.tile([P, 4, 2, N], FP8)
              x8l_sb = xpool.tile([P, 4, 2, N], FP8)
              wf8 = wf8pool.tile([P, 4, 2, 4 * D], FP8)
              nc.sync.dma_start(out=wf8[:, :, :, 0:D], in_=wk8_d[:, :, :, :])
              for sq in range(NCK):
                  nsq = 512 * sq
                  nc.sync.dma_start(out=x8_sb[:, :, :, nsq:nsq + 512],
                                    in_=x8_d[:, :, :, nsq:nsq + 512])
                  nc.sync.dma_start(out=x8l_sb[:, :, :, nsq:nsq + 512],
                                    in_=x8l_d[:, :, :, nsq:nsq + 512])

              # ============ small parameter loads ============
              i128_sb = cpool.tile([P, P], BF16)
              nc.gpsimd.dma_start(out=i128_sb[:, :], in_=i128_d[:, :])
              sel2_sb = cpool.tile([P, CH, P], BF16)
              nc.gpsimd.dma_start(out=sel2_sb[:, :, :], in_=sel2_d[:, :, :])
              gt_sb = cpool.tile([P, TOEP_M, H, P], BF16)
              nc.gpsimd.dma_start(out=gt_sb[:, :, :, :], in_=gt_d[:, :, :, :])
              wrep_sb = cpool.tile([P, NOFF], F32)
              nc.gpsimd.dma_start(out=wrep_sb[:, :], in_=wrep_d[:, :])
              gcpl_sb = cpool.tile([P, P], BF16)
              nc.gpsimd.dma_start(out=gcpl_sb[:, :], in_=gcpl_d[:, :])
              bk_sb = cpool.tile([P, CH], F32)
              nc.gpsimd.dma_start(out=bk_sb[:, :], in_=bk_d[:, :])
              bv_sb = cpool.tile([P, CH], F32)
              nc.gpsimd.dma_start(out=bv_sb[:, :], in_=bv_d[:, :])
              bg_sb = cpool.tile([P, CH], F32)
              nc.gpsimd.dma_start(out=bg_sb[:, :], in_=bg_d[:, :])
              if not ob_zero:
                  ob_sb = cpool.tile([P, D], F32)
                  nc.gpsimd.dma_start(out=ob_sb[:, :], in_=ob_d[:, :])

              # ============ k phase helper: kmag_rep[128, N] ============
              kmag_rep = cpool.tile([P, N], BF16)

              def kphase(s):
                  ns = 512 * s
                  km_ps = tppool.tile([P, 512], F32, tag="tp")
                  for kc in range(CH):
                      ps = pspool.tile([P, 512], F32, tag="mm")
                      for ic in range(4):
                          nc.tensor.matmul(
                              ps[:, :],
                              lhsT=wf8[:, ic, :, P * kc:P * (kc + 1)],
                              rhs=x8_sb[:, ic, :, ns:ns + 512],
                              perf_mode=mybir.MatmulPerfMode.DoubleRow,
                              start=(ic == 0), stop=(ic == 3),
                          )
                      k2 = k2pool.tile([P, 512], BF16, tag="k2")
                      nc.scalar.activation(
                          out=k2[:, :], in_=ps[:, :],
                          func=mybir.ActivationFunctionType.Square,
                          bias=bk_sb[:, kc:kc + 1], scale=1.0 / 1024.0,
                      )
                      nc.tensor.matmul(
                          km_ps[:, :],
                          lhsT=sel2_sb[:, kc, :], rhs=k2[:, :],
                          start=(kc == 0), stop=(kc == CH - 1),
                      )
                  nc.scalar.activation(
                      out=kmag_rep[:, ns:ns + 512], in_=km_ps[:, :],
                      func=mybir.ActivationFunctionType.Sqrt,
                  )

              # ============ weights for v ============
              nc.sync.dma_start(out=wf8[:, :, :, 2 * D:3 * D], in_=wv8_d[:, :, :, :])
              nc.sync.dma_start(out=wf8[:, :, :, 3 * D:4 * D], in_=wv8l_d[:, :, :, :])
              nc.sync.dma_start(out=wf8[:, :, :, D:2 * D], in_=wg8_d[:, :, :, :])
              wo_sb = wopool.tile([P, CH, D], BF16)
              for ic in range(CH):
                  nc.sync.dma_start(out=wo_sb[:, ic, :], in_=wo_d[P * ic:P * (ic + 1), :])

              field = fpool.tile([P, CH, N], BF16)
              acc = apool.tile([P, CH, N], BF16)
              ftm = [None] * NT   # token-major field tiles (ring)
              atm = [None] * NT   # token-major conv acc tiles (ring, d-major)
              gate_ring = [None] * NCK

              def tail(s):
                  ns = 512 * s
                  g_sb = gate_ring[s]
                  for vc in range(CH):
                      ps = pspool.tile([P, 512], F32, tag="mm")
                      nc.tensor.matmul(
                          ps[:, :], lhsT=gcpl_sb[:, :], rhs=acc[:, vc, ns:ns + 512],
                          start=True, stop=True,
                      )
                      # gated = coupled * gate, in place into the gate ring
                      nc.vector.tensor_mul(
                          g_sb[:, vc, :], ps[:, :], g_sb[:, vc, :],
                      )
                  for nt in range(4 * s, 4 * s + 4):
                      nl = P * (nt - 4 * s)
                      ystg = ypool.tile([P, D], BF16, tag="y")
                      for fch in range(2):
                          fs = 512 * fch
                          ps = pspool.tile([P, 512], F32, tag="mm")
                          for vc in range(CH):
                              nc.tensor.matmul(
                                  ps[:, :],
                                  lhsT=g_sb[:, vc, nl:nl + P],
                                  rhs=wo_sb[:, vc, fs:fs + 512],
                                  start=(vc == 0), stop=(vc == CH - 1),
                              )
                          if ob_zero:
                              nc.scalar.activation(
                                  out=ystg[:, fs:fs + 512], in_=ps[:, :],
                                  func=mybir.ActivationFunctionType.Copy,
                              )
                          else:
                              nc.vector.tensor_add(
                                  ystg[:, fs:fs + 512], ps[:, :],
                                  ob_sb[:, fs:fs + 512],
                              )
                      nc.sync.dma_start(out=y_d[P * nt:P * (nt + 1), :], in_=ystg[:, :])

              for s in range(NCK):
                  ns = 512 * s
                  kphase(s)
                  # ---- v matmuls + field (channel-major) for this segment --
                  for vc in range(CH):
                      ps = pspool.tile([P, 512], F32, tag="mm")
                      passes = [(2 * D, x8_sb), (3 * D, x8_sb), (2 * D, x8l_sb)]
                      for pi, (wbase, xop) in enumerate(passes):
                          for ic in range(4):
                              nc.tensor.matmul(
                                  ps[:, :],
                                  lhsT=wf8[:, ic, :, wbase + P * vc:wbase + P * (vc + 1)],
                                  rhs=xop[:, ic, :, ns:ns + 512],
                                  perf_mode=mybir.MatmulPerfMode.DoubleRow,
                                  start=(pi == 0 and ic == 0),
                                  stop=(pi == 2 and ic == 3),
                              )
                      nc.vector.scalar_tensor_tensor(
                          out=field[:, vc, ns:ns + 512],
                          in0=ps[:, :], scalar=bv_sb[:, vc:vc + 1],
                          in1=kmag_rep[:, ns:ns + 512],
                          op0=mybir.AluOpType.add, op1=mybir.AluOpType.mult,
                      )
                  # ---- per 128-token tile: transpose + toeplitz conv ----
                  # software-pipelined: ttm(T+1) and toep(T) interleave so
                  # the PSUM->SBUF drains overlap PE work
                  def ttm(T):
                      ts0 = P * T
                      pst = tppool.tile([P, HD, H], BF16, tag="tp")
                      for vc in range(CH):
                          nc.tensor.transpose(
                              out=pst[:, CH * vc:CH * (vc + 1), :],
                              in_=field[:, vc, ts0:ts0 + P],
                              identity=i128_sb[:, :],
                          )
                      f_tm = ftmpool.tile([P, HD, H], BF16, tag="ftm")
                      ftm[T] = f_tm
                      nc.scalar.activation(
                          out=f_tm[:, :, :], in_=pst[:, :, :],
                          func=mybir.ActivationFunctionType.Copy,
                      )

                  def toep(T):
                      # toeplitz: psum cols head-major [h, d]
                      ps2 = tppool.tile([P, H, HD], F32, tag="tp")
                      for h in range(H):
                          ms = list(range(min(T + 1, TOEP_M)))
                          for m in ms:
                              nc.tensor.matmul(
                                  ps2[:, h, :],
                                  lhsT=gt_sb[:, m, h, :],
                                  rhs=ftm[T - m][:, :, h],
                                  start=(m == ms[0]), stop=(m == ms[-1]),
                              )
                      a_tm = atmpool.tile([P, HD, H], BF16, tag="atm")
                      atm[T] = a_tm
                      # reorder (h, d) -> (d, h) during the PSUM drain
                      nc.scalar.activation(
                          out=a_tm[:, :, :],
                          in_=ps2[:, :, :].transpose([0, 2, 1]),
                          func=mybir.ActivationFunctionType.Copy,
                      )

                  ttm(4 * s)
                  for T in range(4 * s, 4 * s + 4):
                      if T + 1 < 4 * s + 4:
                          ttm(T + 1)
                      toep(T)
                  # ---- transpose back to channel-major acc ----
                  for vc in range(CH):
                      pstb = tppool.tile([P, 512], BF16, tag="tp")
                      for ti in range(4):
                          T = 4 * s + ti
                          nc.tensor.transpose(
                              out=pstb[:, P * ti:P * (ti + 1)],
                              in_=atm[T][:, CH * vc:CH * (vc + 1), :],
                              identity=i128_sb[:, :],
                          )
                      nc.scalar.activation(
                          out=acc[:, vc, ns:ns + 512], in_=pstb[:, :],
                          func=mybir.ActivationFunctionType.Copy,
                      )
                  # ---- large offsets: shifted MACs on VectorE (the Pool
                  # engine rejects TensorScalarPtr on real TRN2) ----
                  def macs(s2, vcs):
                      ns2 = 512 * s2
                      for vc in vcs:
                          for oi in DVE_OFFS:
                              o = OFFSETS[oi]
                              lo = max(ns2, o)
                              if lo < ns2 + 512:
                                  nc.vector.scalar_tensor_tensor(
                                      out=acc[:, vc, lo:ns2 + 512],
                                      in0=field[:, vc, lo - o:ns2 + 512 - o],
                                      scalar=wrep_sb[:, oi:oi + 1],
                                      in1=acc[:, vc, lo:ns2 + 512],
                                      op0=mybir.AluOpType.mult,
                                      op1=mybir.AluOpType.add,
                                  )
                  macs(s, range(0, 4))
                  # ---- gate for this segment (fp8 DoubleRow) ----
                  g_sb = gpool.tile([P, CH, 512], BF16, tag="g")
                  gate_ring[s] = g_sb
                  for gc in range(CH):
                      ps = pspool.tile([P, 512], F32, tag="mm")
                      for ic in range(4):
                          nc.tensor.matmul(
                              ps[:, :],
                              lhsT=wf8[:, ic, :, D + P * gc:D + P * (gc + 1)],
                              rhs=x8_sb[:, ic, :, ns:ns + 512],
                              perf_mode=mybir.MatmulPerfMode.DoubleRow,
                              start=(ic == 0), stop=(ic == 3),
                          )
                      nc.scalar.activation(
                          out=g_sb[:, gc, :], in_=ps[:, :],
                          func=mybir.ActivationFunctionType.Sigmoid,
                          bias=bg_sb[:, gc:gc + 1], scale=1.0 / 1024.0,
                      )
                  macs(s, range(4, CH))
                  if s >= 1:
                      tail(s - 1)
              tail(NCK - 1)

    nc.compile()
    return nc


def _prep_shared(qkv_w, qkv_b, out_w, out_b, gate_w, gate_b, scale_gain, field_coupling):
    perm = PERM
    wk8 = np.ascontiguousarray(
        (32.0 * qkv_w[D:2 * D, :].T).reshape(4, P, 2, D).transpose(1, 0, 2, 3)
        .astype(NP_FP8))
    wv_f = (32.0 * qkv_w[2 * D:3 * D, :][perm, :].T
            .reshape(4, P, 2, D).transpose(1, 0, 2, 3))
    wv8 = wv_f.astype(NP_FP8)
    wv8l = (wv_f - wv8.astype(np.float32)).astype(NP_FP8)
    wg8 = np.ascontiguousarray(
        (32.0 * gate_w[perm, :].T).reshape(4, P, 2, D).transpose(1, 0, 2, 3)
        .astype(NP_FP8))
    wo = np.ascontiguousarray(out_w[:, perm].T.astype(NP_BF16))
    bk = np.ascontiguousarray(qkv_b[D:2 * D].reshape(CH, P).T.astype(np.float32))
    bv = np.ascontiguousarray(
        1024.0 * qkv_b[2 * D:3 * D][perm].reshape(CH, P).T.astype(np.float32))
    bg = np.ascontiguousarray(gate_b[perm].reshape(CH, P).T.astype(np.float32))
    ob = np.ascontiguousarray(np.broadcast_to(out_b.astype(np.float32), (P, D)))

    # conv weights: w[h, o] = sum_j A_MAP[o, j] * softmax(scale_gain)[j, h]
    sg = scale_gain.astype(np.float64)
    e = np.exp(sg - sg.max(axis=0, keepdims=True))
    gains = e / e.sum(axis=0, keepdims=True)             # [S, H]
    w_all = (A_MAP @ gains).T                            # [H, NOFF]
    omax = TOEP_M * P                         # toeplitz covers o < 4*128
    wband = np.zeros((H, omax + P), dtype=np.float64)
    for oi, o in enumerate(OFFSETS):
        if o < omax:
            wband[:, o] = w_all[:, oi]
    pp, cc = np.meshgrid(np.arange(P), np.arange(P), indexing="ij")
    gs = []
    for m in range(TOEP_M):
        dm = cc - pp + m * P                  # block m: o = c - p + 128m
        gs.append(wband[:, np.clip(dm, 0, omax + P - 1)]
                  * ((dm >= 0) & (dm < omax))[None])
    gt = np.stack(gs, axis=0).transpose(2, 0, 1, 3).astype(NP_BF16)
    wrep = np.ascontiguousarray(
        w_all[np.arange(P) % H, :].astype(np.float32))   # [P, NOFF]

    fc = field_coupling.astype(np.float64)
    ec = np.exp(fc - fc.max(axis=-1, keepdims=True))
    csm = ec / ec.sum(axis=-1, keepdims=True)            # [H, H] softmax rows
    gcpl = np.zeros((P, P), dtype=NP_BF16)
    for r in range(CH):
        # 1/1024 cancels the 32x scaling of both x8 and wv8 exactly
        gcpl[H * r:H * (r + 1), H * r:H * (r + 1)] = (
            csm.T / 1024.0).astype(NP_BF16)

    return {"wk8": wk8, "wv8": wv8, "wv8l": wv8l, "wg8": wg8, "wo": wo, "bk": bk,
            "bv": bv, "bg": bg, "ob": ob, "gt": np.ascontiguousarray(gt),
            "wrep": wrep, "gcpl": np.ascontiguousarray(gcpl)}


def _make_in_maps(x, shared):
    in_maps = []
    for b in range(B):
        m = dict(shared)
        xt = 32.0 * x[b].T.reshape(4, P, 2, N).transpose(1, 0, 2, 3)
        hi = xt.astype(NP_FP8)
        m["x8"] = np.ascontiguousarray(hi)
        m["x8l"] = np.ascontiguousarray(
            (xt - hi.astype(np.float32)).astype(NP_FP8))
        in_maps.append(m)
    return in_maps


def kernel(x, qkv_w, qkv_b, out_w, out_b, gate_w, gate_b, scale_gain,
           field_coupling):
    x = np.asarray(x, dtype=np.float32)
    qkv_w = np.asarray(qkv_w, dtype=np.float32)
    qkv_b = np.asarray(qkv_b, dtype=np.float32)
    out_w = np.asarray(out_w, dtype=np.float32)
    out_b = np.asarray(out_b, dtype=np.float32)
    gate_w = np.asarray(gate_w, dtype=np.float32)
    gate_b = np.asarray(gate_b, dtype=np.float32)
    scale_gain = np.asarray(scale_gain, dtype=np.float32)
    field_coupling = np.asarray(field_coupling, dtype=np.float32)

    ob_zero = not np.any(out_b)
    key = ("nc", ob_zero)
    if key not in _CACHE:
        _CACHE[key] = _build_program(ob_zero=ob_zero)
    nc = _CACHE[key]

    shared = _prep_shared(qkv_w, qkv_b, out_w, out_b, gate_w, gate_b,
                          scale_gain, field_coupling)
    in_maps = _make_in_maps(x, shared)

    res = bass_utils.run_bass_kernel_spmd(nc, in_maps, list(range(NCORES)))
    out = np.stack([np.asarray(res.results[b]["y"]).astype(np.float32)
                    for b in range(B)], axis=0)
    return out
